# revision 12
# baseline (speedup 1.0000x reference)
"""Trainium2 Bass kernel for the sparse 3^3 conv FiLM network (nn_CFE_81475529605505).

Strategy
--------
The 192^3 voxel grid is ~1.4% occupied, so the 27-offset neighbor graph is
subcritical: connected components are tiny.  Whole components are partitioned
across the 8 NeuronCores -> zero cross-core edges, no halos, no collectives.
On each core every sparse conv is a dense "center" GEMM (offset 13 is the
identity map) plus ~4.5k real neighbor pairs.

Per-core column layout:
    [deg>=2 targets | per-offset deg-1 runs | ZCOL | isolated points]
Key tricks:
  * deg-1 pair corrections (~80% of pairs) are ACCUMULATING MATMULS straight
    into the center GEMM's PSUM chunks (the run layout makes each offset's
    gathered sources contiguous and 1:1 with its target columns), so the only
    per-point pass is one fused bias+relu PSUM->SBUF copy, alternated between
    the Scalar and Vector engines.
  * gathers use the SWDGE hardware dma_gather (~1us + 0.34ns/descriptor on
    gpsimd, transfers spread over 4 SWDGE queues) instead of the gpsimd
    indirect_copy ucode (27ns/index).  The conv output prefix is mirrored
    point-major to HBM (PE transpose -> SBUF staging -> one DMA); token id
    for layout column t is (t % 128) * NB + t // 128.  Transpose-mode
    dma_gather is limited to 512 indices/instruction, so gathers are chunked.
  * deg>=2 pairs run per-offset TRANSPOSED GEMMs (pairs on PSUM partitions)
    so their yg mirror needs no extra transpose; one dma_gather permutes
    them to round-major zr, added per-round on the deg-desc-sorted prefix.
  * the all-zero pad column ZCOL sits at col NCONN (inside the chunk-padded
    prefix), so gathers never depend on suffix writes.
Everything is bf16 in SBUF; matmuls accumulate in f32 PSUM.
"""
import os
import sys

sys.path.insert(0, "/opt/trn_rl_repo")

import numpy as np
import ml_dtypes

import concourse.bass as bass  # noqa: F401  (import keeps bass registered)
import concourse.mybir as mybir
import concourse.tile as tile
from concourse import bacc
from concourse.bass_utils import run_bass_kernel_spmd

M = 100000
N = 128
NQ = 16
K = 27
NCORES = 8
CENTER = 13
R_PAD = 12800          # per-core padded rows
DCK = 512              # psum chunk width
GCH = 512              # dma_gather chunk (transpose-mode instruction limit)
HSPLIT = 10            # H-mirror staging DMA splits
NQUE = 4               # SWDGE queues
bf16 = mybir.dt.bfloat16
f32 = mybir.dt.float32

_BF = ml_dtypes.bfloat16

_CACHE = {}


# --------------------------------------------------------------------------
# host-side preprocessing
# --------------------------------------------------------------------------

def _components(tgt, src, n):
    """Connected components by min-label propagation + pointer jumping."""
    lab = np.arange(n, dtype=np.int64)
    for _ in range(64):
        prev = lab.copy()
        np.minimum.at(lab, tgt, prev[src])
        np.minimum.at(lab, src, prev[tgt])
        lab = lab[lab]
        lab = lab[lab]
        if np.array_equal(lab, prev):
            break
    return lab


def _wrap_idx(a, dtype, parts=128):
    """Wrapped index layout for gpsimd/SWDGE gathers: [parts, L/16] with
    unwrapped[i] = wrapped[i % 16 + 16*g, i // 16] for each 16-partition
    group g (all groups identical)."""
    L = len(a)
    assert L % 16 == 0
    w = a.reshape(L // 16, 16).T.astype(dtype)   # [16, L/16]
    return np.tile(w, (parts // 16, 1))


def _prep(nbr):
    """Partition + per-core index tensors.  Returns (cfg, cores)."""
    nbr = np.asarray(nbr)
    assert nbr.shape == (K, M)
    center_identity = bool(np.array_equal(nbr[CENTER], np.arange(M)))
    offs = [k for k in range(K) if not (k == CENTER and center_identity)]

    tgt_k, src_k = {}, {}
    all_t, all_s = [], []
    for k in offs:
        mask = nbr[k] >= 0
        tgt_k[k] = np.nonzero(mask)[0].astype(np.int64)
        src_k[k] = nbr[k][mask].astype(np.int64)
        all_t.append(tgt_k[k])
        all_s.append(src_k[k])
    all_t = np.concatenate(all_t)
    all_s = np.concatenate(all_s)

    lab = _components(all_t, all_s, M)
    order = np.argsort(lab, kind="stable")
    slab = lab[order]
    starts = np.nonzero(np.r_[True, slab[1:] != slab[:-1]])[0]
    sizes = np.diff(np.r_[starts, M])
    comp_order = np.argsort(-sizes, kind="stable")
    loads = np.zeros(NCORES, dtype=np.int64)
    member_lists = [[] for _ in range(NCORES)]
    for ci in comp_order:
        c = int(np.argmin(loads))
        member_lists[c].append(order[starts[ci]:starts[ci] + sizes[ci]])
        loads[c] += sizes[ci]
    assert loads.max() <= R_PAD - 2, f"core overflow: {loads}"

    deg = np.zeros(M, dtype=np.int64)
    k1 = np.full(M, -1, dtype=np.int64)      # offset of a deg-1 point's pair
    for k in offs:
        deg[tgt_k[k]] += 1
        k1[tgt_k[k]] = k
    maxdeg = int(deg.max())

    cores = []
    for c in range(NCORES):
        mpts = np.concatenate(member_lists[c])
        d = deg[mpts]
        b2 = mpts[d >= 2][np.argsort(-d[d >= 2], kind="stable")]
        b1 = mpts[d == 1]
        b0 = mpts[d == 0]
        cnt1 = {k: int((k1[b1] == k).sum()) for k in offs}
        deg2s = np.sort(deg[b2])[::-1] if len(b2) else np.zeros(0, np.int64)
        cores.append(dict(b2=b2, b1=b1, b0=b0, cnt1=cnt1, n2=len(b2),
                          n_ge=[int((deg2s >= r).sum())
                                for r in range(maxdeg + 2)]))

    n2max = max(cd["n2"] for cd in cores)
    D1_OFF = -(-(n2max + 1) // DCK) * DCK        # b2 block, chunk aligned
    rs = {k: max(cd["cnt1"][k] for cd in cores) for k in offs}
    A_pos = {}
    p = D1_OFF
    for k in offs:
        A_pos[k] = p
        p += rs[k]
    NCONN = p
    ZCOL = NCONN                                 # all-zero pad column
    NTC = -(-(NCONN + 1) // DCK) * DCK
    NB = NTC // 128                              # token blocks of the prefix

    # deg>=2 per-offset pair counts
    cnt2 = {k: [0] * NCORES for k in offs}
    for c in range(NCORES):
        cd = cores[c]
        in2 = np.zeros(M, dtype=bool)
        in2[cd["b2"]] = True
        for k in offs:
            cnt2[k][c] = int(in2[tgt_k[k]].sum())
    seg2 = {k: max(cnt2[k]) + 1 for k in offs}
    yoff = {}
    p = 0
    for k in offs:
        yoff[k] = p
        p += seg2[k]
    P2S = p                                       # deg2 srcidx width
    NBY = -(-P2S // 128)                          # yg token blocks
    # srcidx layout: [deg2 by offset (P2S) | deg1 by offset (sum rs)]
    P_SUM = P2S + (NCONN - D1_OFF)
    P_PAD = -(-P_SUM // 128) * 128

    # rounds over the deg-desc-sorted b2 block
    n_round = [max(cd["n_ge"][r + 1] for cd in cores) for r in range(maxdeg)]
    n_round = [n for n in n_round if n > 0]
    r_off = []
    p = 0
    for n in n_round:
        r_off.append(p)
        p += -(-n // 16) * 16
    Z_LEN = p
    Z_PAD = -(-Z_LEN // 128) * 128

    # deg-1 accumulate-piece table: per dst chunk, (k, lo, hi) intersections
    pieces_d1 = []
    for c0 in range(D1_OFF, NTC, DCK):
        c1 = c0 + DCK
        ps = []
        for k in offs:
            if rs[k] == 0:
                continue
            lo = max(A_pos[k], c0)
            hi = min(A_pos[k] + rs[k], c1)
            if lo < hi:
                ps.append((k, lo, hi))
        pieces_d1.append(tuple(ps))
    # deg-2 GEMM windows: (win_lo, width, ((k, a, b), ...)) absolute in [0,P2S)
    pieces_d2 = []
    for w0 in range(0, P2S, DCK):
        w1 = min(w0 + DCK, P2S)
        ps = []
        for k in offs:
            lo = max(yoff[k], w0)
            hi = min(yoff[k] + seg2[k], w1)
            if lo < hi:
                ps.append((k, lo, hi))
        pieces_d2.append((w0, w1 - w0, tuple(ps)))

    HB = NB // HSPLIT
    assert NB % HSPLIT == 0

    def tok_main(col):
        b, p = col // 128, col % 128
        return (b // HB) * HB * 128 + p * HB + (b % HB)

    for c in range(NCORES):
        cd = cores[c]
        b2, b1, b0 = cd["b2"], cd["b1"], cd["b0"]
        nb2 = len(b2)
        order_l = [b2]
        bi = 0
        order_l.append(b0[bi:bi + D1_OFF - nb2]); bi += D1_OFF - nb2
        k1b1 = k1[b1]
        for k in offs:
            sel = b1[k1b1 == k]
            order_l.append(sel)
            padn = rs[k] - len(sel)
            assert bi + padn <= len(b0)
            order_l.append(b0[bi:bi + padn]); bi += padn
        rest = b0[bi:]
        pts = np.concatenate(order_l + [rest])
        assert len(np.concatenate(order_l)) == NCONN
        # column map: hole at ZCOL (= NCONN)
        cols = np.r_[np.arange(NCONN), ZCOL + 1 + np.arange(len(rest))]
        assert len(cols) == 0 or cols[-1] <= R_PAD - 1
        cd["pts"] = pts
        cd["cols"] = cols
        g2l = np.full(M, -1, dtype=np.int64)
        g2l[pts] = cols
        in_core = np.zeros(M, dtype=bool)
        in_core[pts] = True

        srcidx = np.full(P_PAD, ZCOL, dtype=np.int64)
        ypos_t = []
        for k in offs:
            m = in_core[tgt_k[k]]
            lt = g2l[tgt_k[k][m]]
            ls = g2l[src_k[k][m]]
            assert (ls >= 0).all(), "cross-core edge"
            # deg-1 part: targets in run k
            m1 = (lt >= A_pos[k]) & (lt < A_pos[k] + rs[k])
            t1, s1 = lt[m1], ls[m1]
            srcidx[P2S + (t1 - D1_OFF)] = s1
            # deg>=2 part: targets in b2 block
            m2 = lt < nb2
            t2, s2 = lt[m2], ls[m2]
            o = np.argsort(t2, kind="stable")
            t2, s2 = t2[o], s2[o]
            assert len(t2) == cnt2[k][c] and m1.sum() + m2.sum() == m.sum()
            srcidx[yoff[k]:yoff[k] + len(s2)] = s2
            for i in range(len(t2)):
                ypos_t.append((int(t2[i]), yoff[k] + i))
        assert srcidx.max() < NTC
        zero_pos = yoff[offs[0]] + cnt2[offs[0]][c]
        ypos_t.sort()
        pm_all = []
        tcur, rankc = -1, 0
        by_rank = {}
        for t, yp in ypos_t:
            rankc = rankc + 1 if t == tcur else 0
            tcur = t
            by_rank.setdefault(rankc, []).append((t, yp))
        for r, nr in enumerate(n_round):
            nr16 = -(-nr // 16) * 16
            pm = np.full(nr16, zero_pos, dtype=np.int64)
            for t, yp in by_rank.get(r, []):
                assert t < nr
                pm[t] = yp
            pm_all.append(pm)
        perm = np.full(Z_PAD, zero_pos, dtype=np.int64)
        perm[:Z_LEN] = np.concatenate(pm_all)
        cd["srcidx"] = srcidx
        cd["srcidx_w"] = _wrap_idx(tok_main(srcidx), np.int16)
        cd["perm_w"] = _wrap_idx(perm, np.int16)

    cfg = dict(
        offs=tuple(offs),
        D1_OFF=D1_OFF,
        NCONN=NCONN,
        NTC=NTC,
        NB=NB,
        NBY=NBY,
        P2S=P2S,
        P_PAD=P_PAD,
        n_round=tuple(n_round),
        r_off=tuple(r_off),
        Z_PAD=Z_PAD,
        pieces_d1=tuple(pieces_d1),
        pieces_d2=tuple(pieces_d2),
        center_identity=center_identity,
    )
    return cfg, cores


# --------------------------------------------------------------------------
# device program
# --------------------------------------------------------------------------

def _build(cfg):
    D1_OFF = cfg["D1_OFF"]
    NTC = cfg["NTC"]
    NB = cfg["NB"]
    NBY = cfg["NBY"]
    P2S = cfg["P2S"]
    P_PAD = cfg["P_PAD"]
    n_round = cfg["n_round"]
    r_off = cfg["r_off"]
    Z_PAD = cfg["Z_PAD"]
    pieces_d1 = cfg["pieces_d1"]
    pieces_d2 = cfg["pieces_d2"]
    Ident = mybir.ActivationFunctionType.Identity
    Relu = mybir.ActivationFunctionType.Relu
    AO = mybir.AluOpType

    nc = bacc.Bacc("TRN2", target_bir_lowering=False, debug=False,
                   num_devices=NCORES, num_swdge_queues=NQUE)

    x_ext = nc.declare_dram_parameter("x", [128, R_PAD], bf16, isOutput=False)
    xgx_ext = nc.declare_dram_parameter("xgx", [128, P_PAD], bf16, isOutput=False)
    xgc_ext = nc.declare_dram_parameter("xgc", [16, P_PAD], bf16, isOutput=False)
    cond_ext = nc.declare_dram_parameter("cond", [16, R_PAD], bf16, isOutput=False)
    w_exts = [nc.declare_dram_parameter(f"w{i}", [128, K * 128], bf16,
                                        isOutput=False) for i in range(4)]
    wq1_ext = nc.declare_dram_parameter("wq1", [16, K * 64], bf16, isOutput=False)
    wq2_ext = nc.declare_dram_parameter("wq2", [64, 128], bf16, isOutput=False)
    wq3_ext = nc.declare_dram_parameter("wq3", [128, 256], bf16, isOutput=False)
    wq4a_ext = nc.declare_dram_parameter("wq4a", [128, 256], bf16, isOutput=False)
    wq4b_ext = nc.declare_dram_parameter("wq4b", [128, 256], bf16, isOutput=False)
    ident_ext = nc.declare_dram_parameter("ident", [128, 128], bf16,
                                          isOutput=False)
    bias_ext = nc.declare_dram_parameter("biases", [128, 10], f32, isOutput=False)
    srcidx_ext = nc.declare_dram_parameter("srcidx", [128, P_PAD // 16],
                                           mybir.dt.int16, isOutput=False)
    perm_ext = nc.declare_dram_parameter("perm", [128, Z_PAD // 16],
                                         mybir.dt.int16, isOutput=False)
    out_ext = nc.declare_dram_parameter("out", [128, R_PAD], bf16, isOutput=True)

    with tile.TileContext(nc) as tc:
        import contextlib
        with contextlib.ExitStack() as ctx:
            pers = ctx.enter_context(tc.tile_pool(name="pers", bufs=1))
            wpool = ctx.enter_context(tc.tile_pool(name="wpool", bufs=2))
            xgp = ctx.enter_context(tc.tile_pool(name="xgp", bufs=2))
            stg = ctx.enter_context(tc.tile_pool(name="stg", bufs=2))
            ygp = ctx.enter_context(tc.tile_pool(name="ygp", bufs=1))
            zrp = ctx.enter_context(tc.tile_pool(name="zrp", bufs=1))
            tmp = ctx.enter_context(tc.tile_pool(name="tmp", bufs=2))
            outp = ctx.enter_context(tc.tile_pool(name="outp", bufs=3))
            drp = ctx.enter_context(tc.tile_pool(name="drp", bufs=2,
                                                 space="DRAM"))
            # PSUM: pm (3) + pr (2) + qa (2) + qb (1) = 8 banks
            pm = ctx.enter_context(tc.tile_pool(name="pm", bufs=3, space="PSUM"))
            pr = ctx.enter_context(tc.tile_pool(name="pr", bufs=2, space="PSUM"))
            pqa = ctx.enter_context(tc.tile_pool(name="pqa", bufs=2, space="PSUM"))
            pqb = ctx.enter_context(tc.tile_pool(name="pqb", bufs=1, space="PSUM"))

            WQ1 = pers.tile([16, K * 64], bf16, name="WQ1", tag="WQ1")
            WQ2 = pers.tile([64, 128], bf16, name="WQ2", tag="WQ2")
            WQ3 = pers.tile([128, 256], bf16, name="WQ3", tag="WQ3")
            WQ4a = pers.tile([128, 256], bf16, name="WQ4a", tag="WQ4a")
            WQ4b = pers.tile([128, 256], bf16, name="WQ4b", tag="WQ4b")
            IDT = pers.tile([128, 128], bf16, name="IDT", tag="IDT")
            BIAS = pers.tile([128, 10], f32, name="BIAS", tag="BIAS")
            SRCI = pers.tile([128, P_PAD // 16], mybir.dt.int16,
                             name="SRCI", tag="SRCI")
            PERM = pers.tile([128, Z_PAD // 16], mybir.dt.int16,
                             name="PERM", tag="PERM")
            XT = pers.tile([128, R_PAD], bf16, name="XT", tag="XT")
            # CONDB: cond on partitions 0:16 early; beta written per chunk
            # later (WAR dep via chunk-wise accesses)
            CONDB = pers.tile([128, R_PAD], bf16, name="CONDB", tag="CONDB")
            # Q1G: q1 on partitions 0:64 early; gamma written per chunk later
            Q1G = pers.tile([128, R_PAD], bf16, name="Q1G", tag="Q1G")
            B1 = pers.tile([128, R_PAD], bf16, name="B1", tag="B1")
            B2 = pers.tile([128, R_PAD], bf16, name="B2", tag="B2")
            H5 = pers.tile([128, D1_OFF], bf16, name="H5", tag="H5")

            nc.sync.dma_start(WQ1[:], wq1_ext[:])
            nc.sync.dma_start(BIAS[:], bias_ext[:])
            nc.sync.dma_start(IDT[:], ident_ext[:])
            nc.sync.dma_start(SRCI[:], srcidx_ext[:])
            nc.sync.dma_start(PERM[:], perm_ext[:])
            nc.sync.dma_start(CONDB[0:16, :], cond_ext[:])
            xgq = xgp.tile([16, P_PAD], bf16, tag="xg", name="xgq")
            nc.sync.dma_start(xgq[:], xgc_ext[:])
            w0sb = wpool.tile([128, K * 128], bf16, tag="wmain", name="w_c1a")
            nc.scalar.dma_start(w0sb[:], w_exts[0][:])
            xga = xgp.tile([128, P_PAD], bf16, tag="xg", name="xga")
            nc.scalar.dma_start(xga[:], xgx_ext[:])
            for h in range(4):
                nc.scalar.dma_start(XT[:, h * 3200:(h + 1) * 3200],
                                    x_ext[:, h * 3200:(h + 1) * 3200])
            nc.sync.dma_start(WQ2[:], wq2_ext[:])
            nc.sync.dma_start(WQ3[:], wq3_ext[:])
            nc.sync.dma_start(WQ4a[:], wq4a_ext[:])
            nc.sync.dma_start(WQ4b[:], wq4b_ext[:])

            def bias_ap(col, parts=128):
                return BIAS[0:parts, col:col + 1]

            # Global SWDGE instruction counter: Tile rotates 8 DMASW sems in
            # program order, so queue = counter % NQUE keeps each sem on one
            # queue (sem s <-> queue s % NQUE), which the sim enforces.
            qctr = [0]

            def dgather(out_sb, h_dram, idx_tile, L, name):
                """Chunked transpose-mode dma_gather (512 idx / instruction)."""
                for c0 in range(0, L, GCH):
                    w = min(GCH, L - c0)
                    nc.gpsimd.dma_gather(
                        out_sb[:, c0:c0 + w].unsqueeze(1), h_dram[:],
                        idx_tile[:, c0 // 16:(c0 + w) // 16], w, w, 128,
                        elem_step=128, transpose=True,
                        queue_num=qctr[0] % NQUE)
                    qctr[0] += 1

            def copy_ps(dst_ap, ps_ap, bcol, relu, parts, eng):
                """Fused psum->sbuf copy with bias (+relu)."""
                if eng == "act":
                    nc.scalar.activation(dst_ap, ps_ap, Relu if relu else Ident,
                                         bias=bias_ap(bcol, parts))
                else:
                    if relu:
                        nc.vector.tensor_scalar(dst_ap, ps_ap,
                                                bias_ap(bcol, parts), 0.0,
                                                AO.add, AO.max)
                    else:
                        nc.vector.tensor_scalar_add(dst_ap, ps_ap,
                                                    bias_ap(bcol, parts))

            def build_H(dst_sb, name):
                """Mirror dst_sb[:, 0:NTC] point-major into a DRAM tile."""
                h_dram = drp.tile([NTC, 128], bf16, tag="H", name=f"H_{name}")
                HB = NB // HSPLIT
                for half in range(HSPLIT):
                    ht = stg.tile([128, HB * 128], bf16, tag="stage",
                                  name=f"ht_{name}_{half}")
                    pt = pm.tile([128, HB * 128], bf16, tag="pm",
                                 name=f"pt_{name}_{half}")
                    for i in range(HB):
                        b = half * HB + i
                        nc.tensor.matmul(pt[:, i * 128:(i + 1) * 128],
                                         dst_sb[:, b * 128:(b + 1) * 128],
                                         IDT[:], is_transpose=True,
                                         skip_group_check=(i > 0))
                    if half % 2 == 0:
                        nc.scalar.activation(ht[:], pt[:], Ident)
                    else:
                        nc.vector.tensor_copy(ht[:], pt[:])
                    nc.sync.dma_start(
                        h_dram[half * HB * 128:(half + 1) * HB * 128, :],
                        ht[:])
                return h_dram

            def corr2(xg_sb, w_sb, wslice, cout, kin, name):
                """deg>=2 per-offset GEMMs (channel-major) -> yg -> PE
                transpose to YT tokens -> HY -> permuted zr [128, Z_PAD]."""
                yg = ygp.tile([128, NBY * 128], bf16, tag="yg",
                              name=f"yg_{name}")
                if cout < 128:
                    nc.vector.memset(yg[cout:128, :], 0.0)
                if NBY * 128 > P2S:
                    nc.vector.memset(yg[0:cout, P2S:NBY * 128], 0.0)
                for wi, (w0, ww, ps) in enumerate(pieces_d2):
                    psc = pm.tile([cout, DCK], f32, tag="pm",
                                  name=f"psc_{name}_{wi}")
                    for i, (k, a, b) in enumerate(ps):
                        nc.tensor.matmul(psc[0:cout, a - w0:b - w0],
                                         w_sb[0:kin, wslice(k)],
                                         xg_sb[0:kin, a:b],
                                         start=(i == 0), stop=(i == len(ps) - 1))
                    if wi % 2 == 0:
                        nc.scalar.activation(yg[0:cout, w0:w0 + ww],
                                             psc[0:cout, 0:ww], Ident)
                    else:
                        nc.vector.tensor_copy(yg[0:cout, w0:w0 + ww],
                                              psc[0:cout, 0:ww])
                yt = stg.tile([128, NBY * 128], bf16, tag="stage",
                              name=f"yt_{name}")
                for g0 in range(0, NBY, 4):
                    gn = min(4, NBY - g0)
                    pt = pm.tile([128, 512], bf16, tag="pm",
                                 name=f"pt_{name}_{g0}")
                    for i in range(gn):
                        by = g0 + i
                        nc.tensor.matmul(pt[:, i * 128:(i + 1) * 128],
                                         yg[:, by * 128:(by + 1) * 128],
                                         IDT[:], is_transpose=True,
                                         skip_group_check=(i > 0))
                    if (g0 // 4) % 2 == 0:
                        nc.scalar.activation(
                            yt[:, g0 * 128:(g0 + gn) * 128],
                            pt[:, 0:gn * 128], Ident)
                    else:
                        nc.vector.tensor_copy(
                            yt[:, g0 * 128:(g0 + gn) * 128],
                            pt[:, 0:gn * 128])
                zr = zrp.tile([128, Z_PAD], bf16, tag="zr", name=f"zr_{name}")
                for c0 in range(0, Z_PAD, GCH):
                    w = min(GCH, Z_PAD - c0)
                    nc.gpsimd.dma_gather(
                        zr[:, c0:c0 + w].unsqueeze(1), yt[:],
                        PERM[:, c0 // 16:(c0 + w) // 16], w, w, 128,
                        transpose=True, queue_num=qctr[0] % NQUE,
                        sbuf_tokens_per_rank=128,
                        sbuf_free_dim_per_rank=256,
                        sbuf_free_dim_pad_per_rank=0,
                        sbuf_byte_offset=0)
                    qctr[0] += 1
                return zr

            def rounds_add(dst_sb, zr, cout):
                for r, nr in enumerate(n_round):
                    nc.vector.tensor_add(dst_sb[0:cout, 0:nr],
                                         dst_sb[0:cout, 0:nr],
                                         zr[0:cout, r_off[r]:r_off[r] + nr])

            def film_chunk(dst_sb, c0, w, eng):
                e = nc.gpsimd if eng == "gps" else nc.vector
                e.tensor_mul(dst_sb[:, c0:c0 + w], dst_sb[:, c0:c0 + w],
                             CONDB[:, c0:c0 + w])
                e.tensor_add(dst_sb[:, c0:c0 + w], dst_sb[:, c0:c0 + w],
                             Q1G[:, c0:c0 + w])

            def conv_suffix(src_sb, dst_sb, wsb, bcol, relu, name):
                for ci, c0 in enumerate(range(NTC, R_PAD, DCK)):
                    ps = pm.tile([128, DCK], f32, tag="pm",
                                 name=f"ps_{name}_s{c0}")
                    nc.tensor.matmul(ps[:],
                                     wsb[:, CENTER * 128:(CENTER + 1) * 128],
                                     src_sb[:, c0:c0 + DCK],
                                     start=True, stop=True)
                    copy_ps(dst_sb[:, c0:c0 + DCK], ps[:], bcol, relu, 128,
                            "act" if ci % 2 == 0 else "dve")

            def conv_prefix(src_sb, dst_sb, wsb, xg, bcol, relu, name,
                            film=False):
                """Center + deg-1-accumulate + deg-2 path over [0, NTC)."""
                zr = corr2(xg, wsb, lambda k: slice(k * 128, (k + 1) * 128),
                           128, 128, name)
                # b2 block: center only in psum, then zr adds (+ relu fixup)
                for ci, c0 in enumerate(range(0, D1_OFF, DCK)):
                    ps = pm.tile([128, DCK], f32, tag="pm",
                                 name=f"ps_{name}_b{c0}")
                    nc.tensor.matmul(ps[:],
                                     wsb[:, CENTER * 128:(CENTER + 1) * 128],
                                     src_sb[:, c0:c0 + DCK],
                                     start=True, stop=True)
                    copy_ps(dst_sb[:, c0:c0 + DCK], ps[:], bcol, False, 128,
                            "act" if ci % 2 == 0 else "dve")
                rounds_add(dst_sb, zr, 128)
                if relu:
                    nc.vector.tensor_scalar_max(dst_sb[:, 0:D1_OFF],
                                                dst_sb[:, 0:D1_OFF], 0.0)
                if film:
                    film_chunk(dst_sb, 0, D1_OFF, "dve")
                # deg-1 runs: center + accumulating per-offset pieces
                for ci, c0 in enumerate(range(D1_OFF, NTC, DCK)):
                    ps = pr.tile([128, DCK], f32, tag="pr",
                                 name=f"ps_{name}_r{c0}")
                    nc.tensor.matmul(ps[:],
                                     wsb[:, CENTER * 128:(CENTER + 1) * 128],
                                     src_sb[:, c0:c0 + DCK],
                                     start=True, stop=False)
                    ps_list = pieces_d1[(c0 - D1_OFF) // DCK]
                    for i, (k, lo, hi) in enumerate(ps_list):
                        xlo = P2S + (lo - D1_OFF)
                        nc.tensor.matmul(ps[0:128, lo - c0:hi - c0],
                                         wsb[:, k * 128:(k + 1) * 128],
                                         xg[:, xlo:xlo + (hi - lo)],
                                         start=False,
                                         stop=(i == len(ps_list) - 1))
                    copy_ps(dst_sb[:, c0:c0 + DCK], ps[:], bcol, relu, 128,
                            "act" if ci % 2 == 0 else "dve")
                    if film:
                        film_chunk(dst_sb, c0, DCK, "dve")

            # ---------------- Q conv: cond -> q1 (relu) ----------------------
            zr_q = corr2(xgq, WQ1, lambda k: slice(k * 64, (k + 1) * 64),
                         64, 16, "q")
            for ci, c0 in enumerate(range(0, R_PAD, DCK)):
                ps = (pr if D1_OFF <= c0 < NTC else pm).tile(
                    [64, DCK], f32, tag="pr" if D1_OFF <= c0 < NTC else "pm",
                    name=f"ps_q_{c0}")
                in_runs = D1_OFF <= c0 < NTC
                nc.tensor.matmul(ps[0:64, :],
                                 WQ1[:, CENTER * 64:(CENTER + 1) * 64],
                                 CONDB[0:16, c0:c0 + DCK],
                                 start=True, stop=not in_runs)
                if in_runs:
                    ps_list = pieces_d1[(c0 - D1_OFF) // DCK]
                    for i, (k, lo, hi) in enumerate(ps_list):
                        xlo = P2S + (lo - D1_OFF)
                        nc.tensor.matmul(ps[0:64, lo - c0:hi - c0],
                                         WQ1[:, k * 64:(k + 1) * 64],
                                         xgq[:, xlo:xlo + (hi - lo)],
                                         start=False,
                                         stop=(i == len(ps_list) - 1))
                copy_ps(Q1G[0:64, c0:c0 + DCK], ps[0:64, :], 4,
                        c0 >= D1_OFF, 64, "act" if ci % 2 == 0 else "dve")
            rounds_add(Q1G, zr_q, 64)
            nc.vector.tensor_scalar_max(Q1G[0:64, 0:D1_OFF],
                                        Q1G[0:64, 0:D1_OFF], 0.0)

            # ---------------- conv1a: XT -> B1 (relu) ------------------------
            conv_prefix(XT, B1, w0sb, xga, 0, True, "c1a")
            h1 = build_H(B1, "c1a")

            w1sb = wpool.tile([128, K * 128], bf16, tag="wmain", name="w_c1b")
            nc.sync.dma_start(w1sb[:], w_exts[1][:])

            # ---------------- gather for conv1b ------------------------------
            xg1 = xgp.tile([128, P_PAD], bf16, tag="xg", name="xg1")
            dgather(xg1, h1, SRCI, P_PAD, "g1b")

            conv_suffix(XT, B1, w0sb, 0, True, "c1a")

            # ---------------- Q MLP: q1 -> beta (CONDB) / gamma (Q1G) --------
            for ci, c0 in enumerate(range(0, R_PAD, DCK)):
                ps2 = pqa.tile([128, DCK], f32, tag="qa", name=f"ps2_{c0}")
                nc.tensor.matmul(ps2[:], WQ2[:], Q1G[0:64, c0:c0 + DCK],
                                 start=True, stop=True)
                q2c = tmp.tile([128, DCK], bf16, tag="q2c", name=f"q2c_{c0}")
                copy_ps(q2c[:], ps2[:], 5, True, 128,
                        "act" if ci % 2 == 0 else "dve")
                ps3a = pqa.tile([128, DCK], f32, tag="qa", name=f"ps3a_{c0}")
                ps3b = pqa.tile([128, DCK], f32, tag="qa", name=f"ps3b_{c0}")
                nc.tensor.matmul(ps3a[:], WQ3[:, 0:128], q2c[:],
                                 start=True, stop=True)
                nc.tensor.matmul(ps3b[:], WQ3[:, 128:256], q2c[:],
                                 start=True, stop=True)
                q3a = tmp.tile([128, DCK], bf16, tag="q3a", name=f"q3a_{c0}")
                q3b = tmp.tile([128, DCK], bf16, tag="q3b", name=f"q3b_{c0}")
                copy_ps(q3a[:], ps3a[:], 6, True, 128,
                        "act" if ci % 2 == 0 else "dve")
                copy_ps(q3b[:], ps3b[:], 7, True, 128,
                        "dve" if ci % 2 == 0 else "act")
                psB = pqb.tile([128, DCK], f32, tag="qb", name=f"psB_{c0}")
                psG = pqb.tile([128, DCK], f32, tag="qb", name=f"psG_{c0}")
                nc.tensor.matmul(psB[:], WQ4a[:, 0:128], q3a[:],
                                 start=True, stop=False)
                nc.tensor.matmul(psB[:], WQ4b[:, 0:128], q3b[:],
                                 start=False, stop=True)
                nc.tensor.matmul(psG[:], WQ4a[:, 128:256], q3a[:],
                                 start=True, stop=False)
                nc.tensor.matmul(psG[:], WQ4b[:, 128:256], q3b[:],
                                 start=False, stop=True)
                copy_ps(CONDB[:, c0:c0 + DCK], psB[:], 8, False, 128,
                        "act" if ci % 2 == 0 else "dve")
                copy_ps(Q1G[:, c0:c0 + DCK], psG[:], 9, False, 128,
                        "dve" if ci % 2 == 0 else "act")

            # ---------------- conv1b: B1 -> B2 + FiLM ------------------------
            conv_prefix(B1, B2, w1sb, xg1, 1, False, "c1b", film=True)
            h2 = build_H(B2, "c1b")
            conv_suffix(B1, B2, w1sb, 1, False, "c1b")

            w2sb = wpool.tile([128, K * 128], bf16, tag="wmain", name="w_c2a")
            nc.sync.dma_start(w2sb[:], w_exts[2][:])

            # ---------------- gather for conv2a ------------------------------
            xg2 = xgp.tile([128, P_PAD], bf16, tag="xg", name="xg2")
            dgather(xg2, h2, SRCI, P_PAD, "g2a")

            # suffix FiLM lags the gather (DVE, low priority)
            for c0 in range(NTC, R_PAD, DCK):
                film_chunk(B2, c0, DCK, "dve")

            # ---------------- conv2a: B2 -> B1 (relu) ------------------------
            conv_prefix(B2, B1, w2sb, xg2, 2, True, "c2a")
            h3 = build_H(B1, "c2a")
            conv_suffix(B2, B1, w2sb, 2, True, "c2a")

            w3sb = wpool.tile([128, K * 128], bf16, tag="wmain", name="w_c2b")
            nc.sync.dma_start(w3sb[:], w_exts[3][:])

            # ---------------- gather for conv2b ------------------------------
            xg3 = xgp.tile([128, P_PAD], bf16, tag="xg", name="xg3")
            dgather(xg3, h3, SRCI, P_PAD, "g2b")

            # ---------------- conv2b + residual -> out -----------------------
            def out_chunk(c0, w, src_ap, name):
                oc = outp.tile([128, DCK], bf16, tag="oc", name=name)
                nc.vector.scalar_tensor_tensor(
                    oc[:, 0:w], src_ap, bias_ap(3), XT[:, c0:c0 + w],
                    AO.add, AO.add)
                nc.sync.dma_start(out_ext[:, c0:c0 + w], oc[:, 0:w])

            zr5 = corr2(xg3, w3sb, lambda k: slice(k * 128, (k + 1) * 128),
                        128, 128, "c2b")
            for ci, c0 in enumerate(range(0, D1_OFF, DCK)):
                ps = pm.tile([128, DCK], f32, tag="pm", name=f"ps_c2b_b{c0}")
                nc.tensor.matmul(ps[:], w3sb[:, CENTER * 128:(CENTER + 1) * 128],
                                 B1[:, c0:c0 + DCK], start=True, stop=True)
                copy_ps(H5[:, c0:c0 + DCK], ps[:], 3, False, 128,
                        "act" if ci % 2 == 0 else "dve")
            rounds_add(H5, zr5, 128)
            for c0 in range(0, D1_OFF, DCK):
                oc = outp.tile([128, DCK], bf16, tag="oc", name=f"oc_b{c0}")
                nc.vector.tensor_add(oc[:], H5[:, c0:c0 + DCK],
                                     XT[:, c0:c0 + DCK])
                nc.sync.dma_start(out_ext[:, c0:c0 + DCK], oc[:])
            for c0 in range(D1_OFF, NTC, DCK):
                ps = pr.tile([128, DCK], f32, tag="pr", name=f"ps_c2b_r{c0}")
                nc.tensor.matmul(ps[:], w3sb[:, CENTER * 128:(CENTER + 1) * 128],
                                 B1[:, c0:c0 + DCK], start=True, stop=False)
                ps_list = pieces_d1[(c0 - D1_OFF) // DCK]
                for i, (k, lo, hi) in enumerate(ps_list):
                    xlo = P2S + (lo - D1_OFF)
                    nc.tensor.matmul(ps[0:128, lo - c0:hi - c0],
                                     w3sb[:, k * 128:(k + 1) * 128],
                                     xg3[:, xlo:xlo + (hi - lo)],
                                     start=False, stop=(i == len(ps_list) - 1))
                out_chunk(c0, DCK, ps[:], f"oc_r{c0}")
            for c0 in range(NTC, R_PAD, DCK):
                ps = pm.tile([128, DCK], f32, tag="pm", name=f"ps_c2b_s{c0}")
                nc.tensor.matmul(ps[:], w3sb[:, CENTER * 128:(CENTER + 1) * 128],
                                 B1[:, c0:c0 + DCK], start=True, stop=True)
                out_chunk(c0, DCK, ps[:], f"oc_s{c0}")

    nc.finalize()
    return nc


# --------------------------------------------------------------------------
# entry points
# --------------------------------------------------------------------------

def _get_compiled(cfg):
    key = (cfg["offs"], cfg["P_PAD"], cfg["P2S"], cfg["n_round"],
           cfg["NTC"], cfg["D1_OFF"], cfg["pieces_d1"], cfg["pieces_d2"])
    if key not in _CACHE:
        _CACHE[key] = _build(cfg)
    return _CACHE[key]


def _make_in_maps(inputs, cfg, cores):
    def w_main(W):
        W = np.asarray(W).astype(_BF)
        return np.ascontiguousarray(W.transpose(1, 0, 2).reshape(128, K * 128))

    W1a = w_main(inputs["W1a"]); W1b = w_main(inputs["W1b"])
    W2a = w_main(inputs["W2a"]); W2b = w_main(inputs["W2b"])
    if not cfg["center_identity"]:
        for Wm in (W1a, W1b, W2a, W2b):
            Wm[:, CENTER * 128:(CENTER + 1) * 128] = 0
    WQ1 = np.asarray(inputs["WQ1"]).astype(_BF)
    WQ1 = np.ascontiguousarray(WQ1.transpose(1, 0, 2).reshape(16, K * 64))
    if not cfg["center_identity"]:
        WQ1[:, CENTER * 64:(CENTER + 1) * 64] = 0
    WQ2 = np.ascontiguousarray(np.asarray(inputs["WQ2"]).astype(_BF))
    WQ3 = np.ascontiguousarray(np.asarray(inputs["WQ3"]).astype(_BF))
    WQ4 = np.asarray(inputs["WQ4"]).astype(_BF)
    WQ4a = np.ascontiguousarray(WQ4[0:128])
    WQ4b = np.ascontiguousarray(WQ4[128:256])
    biases = np.zeros((128, 10), np.float32)
    biases[:, 0] = np.asarray(inputs["b1a"])
    biases[:, 1] = np.asarray(inputs["b1b"])
    biases[:, 2] = np.asarray(inputs["b2a"])
    biases[:, 3] = np.asarray(inputs["b2b"])
    biases[0:64, 4] = np.asarray(inputs["bQ1"])
    biases[:, 5] = np.asarray(inputs["bQ2"])
    biases[:, 6] = np.asarray(inputs["bQ3"])[0:128]
    biases[:, 7] = np.asarray(inputs["bQ3"])[128:256]
    biases[:, 8] = np.asarray(inputs["bQ4"])[0:128]
    biases[:, 9] = np.asarray(inputs["bQ4"])[128:256]

    x = np.asarray(inputs["x_feats"]).astype(_BF)
    cond = np.asarray(inputs["cond_feats"]).astype(_BF)

    in_maps = []
    for c in range(NCORES):
        cd = cores[c]
        pts, cols = cd["pts"], cd["cols"]
        xc = np.zeros((128, R_PAD), _BF)
        xc[:, cols] = x[pts].T
        cc16 = np.zeros((16, R_PAD), _BF)
        cc16[:, cols] = cond[pts].T
        si = cd["srcidx"]
        xgx = np.ascontiguousarray(xc[:, si])
        xgc = np.ascontiguousarray(cc16[:, si])
        im = dict(x=xc, cond=cc16, xgx=xgx, xgc=xgc,
                  w0=W1a, w1=W1b, w2=W2a, w3=W2b,
                  wq1=WQ1, wq2=WQ2, wq3=WQ3, wq4a=WQ4a, wq4b=WQ4b,
                  ident=np.eye(128, dtype=_BF),
                  biases=biases, srcidx=cd["srcidx_w"], perm=cd["perm_w"])
        in_maps.append(im)
    return in_maps


def _run(inputs, trace=False):
    """Returns (out [M,128] f32, BassKernelResults)."""
    cfg, cores = _prep(np.asarray(inputs["nbr"]))
    nc = _get_compiled(cfg)
    in_maps = _make_in_maps(inputs, cfg, cores)
    ncores_run = int(os.environ.get("KCORES", str(NCORES)))
    res = run_bass_kernel_spmd(nc, in_maps[:ncores_run],
                               core_ids=list(range(ncores_run)), trace=trace)
    out = np.zeros((M, N), np.float32)
    for c in range(ncores_run):
        pts, cols = cores[c]["pts"], cores[c]["cols"]
        oc = np.asarray(res.results[c]["out"])
        out[pts] = oc[:, cols].T.astype(np.float32)
    return out, res


def kernel(**inputs):
    out, _ = _run(inputs, trace=False)
    return out
